# revision 6
# baseline (speedup 1.0000x reference)
"""Trainium2 Bass kernel for a cross-attention repair block.

Model (B=2, T=2048, D=1024, H=16, DH=64, band<=64):
    Q  = LN(x_refined) @ Wq + bq            (scaled by 1/sqrt(DH), folded on host)
    KV = LN(x_mem);  K = KV@Wk+bk; V = KV@Wv+bv
    banded attention  |t-s| <= band
    y  = x_refined + ctx @ Wo + bo
    out = y + (gelu(LN(y) @ W1 + b1) @ W2 + b2)

Sharding: row-parallel over B*T across 8 cores (512 rows each), zero
collectives.  The banded attention needs only a +-64 row halo of x_mem,
which the host pads with zeros; masks are built host-side from `band`
and shipped as an input tensor, so the same NEFF runs on all cores.
"""

import sys

if "/opt/trn_rl_repo" not in sys.path:
    sys.path.insert(0, "/opt/trn_rl_repo")

import numpy as np

# ----------------------------------------------------------------------------
# problem constants (hardcoded; kernel.py must be self-contained)
B, T, D, H = 2, 2048, 1024, 16
DH = D // H                     # 64
FF = 4 * D                      # 4096
P = 128                         # partitions
KD = D // P                     # 8 feature tiles
MT = FF // P                    # 32 FFN1 m-tiles
NCORES = 8
CPB = NCORES // B               # cores per batch = 4
R = T // CPB                    # 512 rows per core
NR = R // P                     # 4 row tiles
HALO = 64                       # key-window halo (supports band <= 64)
RH = R + 2 * HALO               # 640 x_mem rows per core
NRH = RH // P                   # 5
WIN = 256                       # key window per q-tile (2 key tiles)
EPS = 1e-5
NEG = -30000.0

_CACHE = {}


# ----------------------------------------------------------------------------
def _build():
    """Build + compile the per-core Bass module. Returns (nc, input names)."""
    from contextlib import ExitStack

    import concourse.bass as bass
    import concourse.mybir as mybir
    import concourse.tile as tile
    from concourse import bacc
    from concourse.masks import make_identity

    fp32 = mybir.dt.float32
    Alu = mybir.AluOpType
    Act = mybir.ActivationFunctionType

    nc = bacc.Bacc("TRN2", target_bir_lowering=False, debug=False,
                   enable_asserts=False)

    # ---- DRAM I/O ----------------------------------------------------------
    d_xr = nc.dram_tensor("xr", [R, D], fp32, kind="ExternalInput").ap()
    d_xm = nc.dram_tensor("xm", [RH, D], fp32, kind="ExternalInput").ap()
    d_maskT = nc.dram_tensor("maskT", [P, NR, 2, P], fp32,
                             kind="ExternalInput").ap()
    d_Wq = nc.dram_tensor("Wq", [D, D], fp32, kind="ExternalInput").ap()
    d_Wk = nc.dram_tensor("Wk", [D, D], fp32, kind="ExternalInput").ap()
    d_Wv = nc.dram_tensor("Wv", [D, D], fp32, kind="ExternalInput").ap()
    d_Wo = nc.dram_tensor("Wo", [D, D], fp32, kind="ExternalInput").ap()
    d_W1 = nc.dram_tensor("W1", [D, FF], fp32, kind="ExternalInput").ap()
    d_W2 = nc.dram_tensor("W2", [FF, D], fp32, kind="ExternalInput").ap()
    # per-partition params, host pre-arranged as [128, n]
    d_bq = nc.dram_tensor("bq_pp", [P, KD], fp32, kind="ExternalInput").ap()
    d_bk = nc.dram_tensor("bk_pp", [P, KD], fp32, kind="ExternalInput").ap()
    d_b1 = nc.dram_tensor("b1_pp", [P, MT], fp32, kind="ExternalInput").ap()
    d_lnq_g = nc.dram_tensor("lnq_g", [P, KD], fp32, kind="ExternalInput").ap()
    d_lnq_b = nc.dram_tensor("lnq_b", [P, KD], fp32, kind="ExternalInput").ap()
    d_lnkv_g = nc.dram_tensor("lnkv_g", [P, KD], fp32, kind="ExternalInput").ap()
    d_lnkv_b = nc.dram_tensor("lnkv_b", [P, KD], fp32, kind="ExternalInput").ap()
    d_lnf_g = nc.dram_tensor("lnf_g", [P, KD], fp32, kind="ExternalInput").ap()
    d_lnf_b = nc.dram_tensor("lnf_b", [P, KD], fp32, kind="ExternalInput").ap()
    # free-dim biases, broadcast across partitions at load time
    d_bv = nc.dram_tensor("bv", [D], fp32, kind="ExternalInput").ap()
    d_bo = nc.dram_tensor("bo", [D], fp32, kind="ExternalInput").ap()
    d_b2 = nc.dram_tensor("b2", [D], fp32, kind="ExternalInput").ap()
    d_out = nc.dram_tensor("out", [R, D], fp32, kind="ExternalOutput").ap()

    def bcast128(ap1d):
        return bass.AP(tensor=ap1d.tensor, offset=ap1d.offset,
                       ap=[[0, P]] + list(ap1d.ap))

    with tile.TileContext(nc) as tc, ExitStack() as top:
        consts = top.enter_context(tc.tile_pool(name="consts", bufs=1))
        lnstage = top.enter_context(tc.tile_pool(name="lnstage", bufs=3))
        smtmp = top.enter_context(tc.tile_pool(name="smtmp", bufs=1))
        actT = top.enter_context(tc.tile_pool(name="actT", bufs=1))
        ypool = top.enter_context(tc.tile_pool(name="ypool", bufs=1))

        # ---- constants -----------------------------------------------------
        identity = consts.tile([P, P], fp32, name="identity")
        make_identity(nc, identity)
        ones_col = consts.tile([P, 1], fp32, name="ones_col")
        nc.vector.memset(ones_col, 1.0)
        eps_t = consts.tile([P, 1], fp32, name="eps_t")
        nc.vector.memset(eps_t, EPS)
        maskT = consts.tile([P, NR, 2, P], fp32, name="maskT_sb")
        nc.sync.dma_start(out=maskT, in_=d_maskT)

        def ld_pp(apd, n, nm):
            t = consts.tile([P, n], fp32, name=nm)
            nc.sync.dma_start(out=t, in_=apd)
            return t

        bq_pp = ld_pp(d_bq, KD, "bq_sb")
        bk_pp = ld_pp(d_bk, KD, "bk_sb")
        b1_pp = ld_pp(d_b1, MT, "b1_sb")
        lnq_g = ld_pp(d_lnq_g, KD, "lnqg_sb")
        lnq_b = ld_pp(d_lnq_b, KD, "lnqb_sb")
        lnkv_g = ld_pp(d_lnkv_g, KD, "lnkvg_sb")
        lnkv_b = ld_pp(d_lnkv_b, KD, "lnkvb_sb")
        lnf_g = ld_pp(d_lnf_g, KD, "lnfg_sb")
        lnf_b = ld_pp(d_lnf_b, KD, "lnfb_sb")

        def ld_bc(apd, nm):
            t = consts.tile([P, D], fp32, name=nm)
            nc.gpsimd.dma_start(out=t, in_=bcast128(apd))
            return t

        bv_bc = ld_bc(d_bv, "bv_bc")
        bo_bc = ld_bc(d_bo, "bo_bc")
        b2_bc = ld_bc(d_b2, "b2_bc")

        # ---- persistent activations ---------------------------------------
        # LNxrT and CtxT share one slot (disjoint lifetimes)
        LNxrT = actT.tile([P, KD, R], fp32, tag="actT", name="LNxrT")
        y_sb = ypool.tile([P, NR, D], fp32, name="y_sb")

        es_xa = ExitStack()  # xr lives until after O-proj
        poolXA = es_xa.enter_context(tc.tile_pool(name="poolXA", bufs=1))
        xr_sb = poolXA.tile([P, NR, D], fp32, name="xr_sb")
        nc.sync.dma_start(out=xr_sb, in_=d_xr.rearrange("(n p) d -> p n d", p=P))

        es_w = ExitStack()  # streamed projection weights (quarter-W chunks)
        wproj = es_w.enter_context(tc.tile_pool(name="wproj", bufs=2))

        # right-side SBUF stack: poolQKV under poolEarly (LIFO-compatible:
        # poolEarly closes after V, poolQKV closes after attention)
        es_qkv = ExitStack()
        poolQKV = es_qkv.enter_context(
            tc.tile_pool(name="poolQKV", bufs=1, side="right"))
        QT = poolQKV.tile([P, KD, R], fp32, name="QT")
        KT = poolQKV.tile([P, KD, RH], fp32, name="KT")
        V = poolQKV.tile([P, NRH, D], fp32, name="V")

        es_early = ExitStack()  # x_mem + LNxmT die after K/V projections
        poolEarly = es_early.enter_context(
            tc.tile_pool(name="poolEarly", bufs=1, side="right"))
        xm_sb = poolEarly.tile([P, NRH, D], fp32, name="xm_sb")
        nc.sync.dma_start(out=xm_sb, in_=d_xm.rearrange("(n p) d -> p n d", p=P))
        LNxmT = poolEarly.tile([P, KD, RH], fp32, name="LNxmT")

        # ---- helpers -------------------------------------------------------
        def ln_transpose(x_nat, n_tiles, xT_out, g_pp, b_pp, trp, nm):
            """LayerNorm along D in natural layout, emit transposed
            (feature-major) output with gamma/beta fused per-partition."""
            for rt in range(n_tiles):
                xrow = x_nat[:, rt, :]
                stats = lnstage.tile([P, 2, 6], fp32, tag="ln_stats",
                                     name=f"st_{nm}{rt}")
                nc.vector.bn_stats(out=stats[:, 0, :], in_=xrow[:, 0:512])
                nc.vector.bn_stats(out=stats[:, 1, :], in_=xrow[:, 512:1024])
                mv = lnstage.tile([P, 2], fp32, tag="ln_mv", name=f"mv_{nm}{rt}")
                nc.vector.bn_aggr(out=mv, in_=stats)
                # mv[:,1] = 1/sqrt(var+eps)
                nc.scalar.activation(out=mv[:, 1:2], in_=mv[:, 1:2],
                                     func=Act.Sqrt, bias=eps_t, scale=1.0)
                nc.vector.reciprocal(out=mv[:, 1:2], in_=mv[:, 1:2])
                xc = lnstage.tile([P, D], fp32, tag="ln_xc", name=f"xc_{nm}{rt}")
                nc.vector.tensor_scalar(out=xc, in0=xrow,
                                        scalar1=mv[:, 0:1], scalar2=mv[:, 1:2],
                                        op0=Alu.subtract, op1=Alu.mult)
                for f in range(KD):
                    pt = trp.tile([P, P], fp32, tag="tr", name=f"tr_{nm}{rt}{f}")
                    nc.tensor.transpose(pt, xc[:, f * P:(f + 1) * P], identity)
                    nc.vector.tensor_scalar(
                        out=xT_out[:, f, rt * P:(rt + 1) * P], in0=pt,
                        scalar1=g_pp[:, f:f + 1], scalar2=b_pp[:, f:f + 1],
                        op0=Alu.mult, op1=Alu.add)

        def load_wq(dram_w, quarter, nm, cols=D):
            """Load W[:, quarter*256 : ...+256] as [128, KD, 256] chunk."""
            w = wproj.tile([P, KD, 256], fp32, tag="wproj", name=nm)
            src = dram_w.rearrange("(k p) m -> p k m", p=P)
            nc.sync.dma_start(out=w, in_=src[:, :, quarter * 256:(quarter + 1) * 256])
            return w

        # =====================================================================
        # scope 1: LayerNorms + Q/K/V projections
        # =====================================================================
        with tc.tile_pool(name="trp", bufs=2, space="PSUM") as trp, \
             tc.tile_pool(name="mmp", bufs=2, space="PSUM") as mmp:

            ln_transpose(xr_sb, NR, LNxrT, lnq_g, lnq_b, trp, "xr")
            ln_transpose(xm_sb, NRH, LNxmT, lnkv_g, lnkv_b, trp, "xm")

            # QT[m,:] = (Wq^T @ LNxr^T)[m] + bq[m]   (m = output-feature tile)
            for q in range(4):
                wq = load_wq(d_Wq, q, f"wq{q}")
                for ms in range(2):
                    m = q * 2 + ms
                    ps = mmp.tile([P, R], fp32, tag="mm", name=f"psq{m}")
                    for k in range(KD):
                        nc.tensor.matmul(ps, lhsT=wq[:, k, ms * P:(ms + 1) * P],
                                         rhs=LNxrT[:, k, :],
                                         start=(k == 0), stop=(k == KD - 1))
                    nc.vector.tensor_scalar(out=QT[:, m, :], in0=ps,
                                            scalar1=bq_pp[:, m:m + 1],
                                            scalar2=None, op0=Alu.add)

            # KT (640 rows -> N chunks of 512+128)
            for q in range(4):
                wk = load_wq(d_Wk, q, f"wk{q}")
                for ms in range(2):
                    m = q * 2 + ms
                    for n0, nsz in ((0, 512), (512, 128)):
                        ps = mmp.tile([P, nsz], fp32, tag="mm", name=f"psk{m}_{n0}")
                        for k in range(KD):
                            nc.tensor.matmul(ps, lhsT=wk[:, k, ms * P:(ms + 1) * P],
                                             rhs=LNxmT[:, k, n0:n0 + nsz],
                                             start=(k == 0), stop=(k == KD - 1))
                        nc.vector.tensor_scalar(out=KT[:, m, n0:n0 + nsz], in0=ps,
                                                scalar1=bk_pp[:, m:m + 1],
                                                scalar2=None, op0=Alu.add)

            # V natural: V[rt, n] = (LNxm @ Wv)[rt, n] + bv[n]
            for q in range(4):
                wv = load_wq(d_Wv, q, f"wv{q}")
                n0 = q * 256
                for rt in range(NRH):
                    ps = mmp.tile([P, 256], fp32, tag="mm", name=f"psv{q}_{rt}")
                    for k in range(KD):
                        nc.tensor.matmul(ps, lhsT=LNxmT[:, k, rt * P:(rt + 1) * P],
                                         rhs=wv[:, k, :],
                                         start=(k == 0), stop=(k == KD - 1))
                    nc.vector.tensor_add(out=V[:, rt, n0:n0 + 256], in0=ps,
                                         in1=bv_bc[:, n0:n0 + 256])

        es_early.close()  # free x_mem + LNxmT

        # =====================================================================
        # scope 2: banded attention
        # =====================================================================
        CtxT = actT.tile([P, KD, R], fp32, tag="actT", name="CtxT")
        with tc.tile_pool(name="stp", bufs=4, space="PSUM") as stp, \
             tc.tile_pool(name="ctxp", bufs=2, space="PSUM") as ctxp, \
             tc.tile_pool(name="denp", bufs=2, space="PSUM") as denp:
            for h in range(H):
                ft, po = h // 2, 64 * (h % 2)
                ctxh = ctxp.tile([DH, R], fp32, tag="ctxh", name=f"ctx{h}")
                denh = denp.tile([1, R], fp32, tag="denh", name=f"den{h}")
                for qt in range(NR):
                    for kt in range(2):
                        st = stp.tile([P, P], fp32, tag="st", name=f"s{h}_{qt}{kt}")
                        kc = (qt + kt) * P
                        # S^T[s, t] = sum_d K^T[d, s] Q^T[d, t]
                        nc.tensor.matmul(
                            st, lhsT=KT[po:po + DH, ft, kc:kc + P],
                            rhs=QT[po:po + DH, ft, qt * P:(qt + 1) * P],
                            start=True, stop=True)
                        nc.vector.tensor_add(out=st, in0=st,
                                             in1=maskT[:, qt, kt, :])
                        es = smtmp.tile([P, P], fp32, tag="es", bufs=6,
                                        name=f"e{h}_{qt}{kt}")
                        nc.scalar.activation(out=es, in_=st, func=Act.Exp)
                        # ctx^T[d, t] += sum_s V[s, d] * expS^T[s, t]
                        nc.tensor.matmul(
                            ctxh[:, qt * P:(qt + 1) * P],
                            lhsT=V[:, qt + kt, h * DH:(h + 1) * DH], rhs=es,
                            start=(kt == 0), stop=(kt == 1))
                        nc.tensor.matmul(
                            denh[:, qt * P:(qt + 1) * P],
                            lhsT=ones_col, rhs=es,
                            start=(kt == 0), stop=(kt == 1))
                rh = smtmp.tile([1, R], fp32, tag="rh", bufs=3, name=f"r{h}")
                nc.vector.reciprocal(out=rh, in_=denh)
                rb = smtmp.tile([DH, R], fp32, tag="rb", bufs=3, name=f"rb{h}")
                nc.gpsimd.partition_broadcast(rb, rh)
                nc.vector.tensor_mul(out=CtxT[po:po + DH, ft, :], in0=ctxh,
                                     in1=rb)

        # =====================================================================
        # scope 3: O-projection + residual, LN(y), FFN1 (+gelu)
        # =====================================================================
        with tc.tile_pool(name="trp2", bufs=2, space="PSUM") as trp2, \
             tc.tile_pool(name="mmp2", bufs=2, space="PSUM") as mmp2:
            # y = ctx @ Wo + bo + x_refined
            for q in range(4):
                wo = load_wq(d_Wo, q, f"wo{q}")
                n0 = q * 256
                for rt in range(NR):
                    ps = mmp2.tile([P, 256], fp32, tag="mm2", name=f"pso{q}_{rt}")
                    for k in range(KD):
                        nc.tensor.matmul(ps, lhsT=CtxT[:, k, rt * P:(rt + 1) * P],
                                         rhs=wo[:, k, :],
                                         start=(k == 0), stop=(k == KD - 1))
                    ysl = y_sb[:, rt, n0:n0 + 256]
                    nc.vector.tensor_add(out=ysl, in0=ps, in1=bo_bc[:, n0:n0 + 256])
                    nc.vector.tensor_add(out=ysl, in0=ysl,
                                         in1=xr_sb[:, rt, n0:n0 + 256])

            es_qkv.close()  # free QT/KT/V (right stack fully empty)
            es_w.close()    # free projection weight slots
            es_xa.close()   # free xr

            es_ffn = ExitStack()
            poolFFN = es_ffn.enter_context(tc.tile_pool(name="poolFFN", bufs=1))
            LNyT = poolFFN.tile([P, KD, R], fp32, name="LNyT")
            gT = poolFFN.tile([P, MT, R], fp32, name="gT")

            ln_transpose(y_sb, NR, LNyT, lnf_g, lnf_b, trp2, "y")

            # FFN1: gT[m,:] = gelu((W1^T @ LNy^T)[m] + b1[m])
            es_w2 = ExitStack()
            w1pool = es_w2.enter_context(tc.tile_pool(name="w1pool", bufs=2))
            w2pool = es_w2.enter_context(tc.tile_pool(name="w2pool", bufs=3))
            outst = es_w2.enter_context(tc.tile_pool(name="outst", bufs=3))
            for mb in range(8):
                w1b = w1pool.tile([P, KD, 512], fp32, tag="w1", name=f"w1_{mb}")
                src = d_W1.rearrange("(k p) m -> p k m", p=P)
                nc.sync.dma_start(out=w1b,
                                  in_=src[:, :, mb * 512:(mb + 1) * 512])
                for ms in range(4):
                    m = mb * 4 + ms
                    ps = mmp2.tile([P, R], fp32, tag="mm2", name=f"psf{m}")
                    for k in range(KD):
                        nc.tensor.matmul(ps, lhsT=w1b[:, k, ms * P:(ms + 1) * P],
                                         rhs=LNyT[:, k, :],
                                         start=(k == 0), stop=(k == KD - 1))
                    nc.scalar.activation(out=gT[:, m, :], in_=ps, func=Act.Gelu,
                                         bias=b1_pp[:, m:m + 1], scale=1.0)

        # =====================================================================
        # scope 4: FFN2 (k-major over all 8 PSUM banks) + final residual
        # =====================================================================
        with tc.tile_pool(name="accp", bufs=8, space="PSUM") as accp:
            accs = [accp.tile([P, 512], fp32, tag="acc", name=f"acc{i}")
                    for i in range(8)]
            for k in range(MT):
                w2k = w2pool.tile([P, D], fp32, tag="w2", name=f"w2_{k}")
                nc.sync.dma_start(out=w2k, in_=d_W2[k * P:(k + 1) * P, :])
                for rt in range(NR):
                    for n in range(2):
                        nc.tensor.matmul(
                            accs[rt * 2 + n],
                            lhsT=gT[:, k, rt * P:(rt + 1) * P],
                            rhs=w2k[:, n * 512:(n + 1) * 512],
                            start=(k == 0), stop=(k == MT - 1))
            d_out_t = d_out.rearrange("(n p) d -> p n d", p=P)
            for rt in range(NR):
                for n in range(2):
                    n0 = n * 512
                    t1 = outst.tile([P, 512], fp32, tag="out", name=f"o{rt}{n}")
                    nc.vector.tensor_add(out=t1, in0=accs[rt * 2 + n],
                                         in1=b2_bc[:, n0:n0 + 512])
                    nc.vector.tensor_add(out=t1, in0=t1,
                                         in1=y_sb[:, rt, n0:n0 + 512])
                    nc.sync.dma_start(out=d_out_t[:, rt, n0:n0 + 512], in_=t1)
        es_w2.close()
        es_ffn.close()

    nc.compile()
    return nc


# ----------------------------------------------------------------------------
def _host_inputs(x_refined, x_mem, ln_q_g, ln_q_b, ln_kv_g, ln_kv_b,
                 Wq, bq, Wk, bk, Wv, bv, Wo, bo,
                 ln_f_g, ln_f_b, W1, b1, W2, b2, band):
    """Slice/arrange the full inputs into 8 per-core input maps."""
    band = int(np.asarray(band))
    assert band <= HALO, f"band={band} > supported halo {HALO}"
    f32 = np.float32

    scale = 1.0 / np.sqrt(np.float32(DH))
    Wq_s = np.ascontiguousarray(np.asarray(Wq, f32) * scale)
    bq_s = np.asarray(bq, f32) * scale

    def pp(v, n):
        return np.ascontiguousarray(np.asarray(v, f32).reshape(n, P).T)

    common = {
        "Wq": Wq_s, "Wk": np.ascontiguousarray(np.asarray(Wk, f32)),
        "Wv": np.ascontiguousarray(np.asarray(Wv, f32)),
        "Wo": np.ascontiguousarray(np.asarray(Wo, f32)),
        "W1": np.ascontiguousarray(np.asarray(W1, f32)),
        "W2": np.ascontiguousarray(np.asarray(W2, f32)),
        "bq_pp": pp(bq_s, KD), "bk_pp": pp(bk, KD), "b1_pp": pp(b1, MT),
        "lnq_g": pp(ln_q_g, KD), "lnq_b": pp(ln_q_b, KD),
        "lnkv_g": pp(ln_kv_g, KD), "lnkv_b": pp(ln_kv_b, KD),
        "lnf_g": pp(ln_f_g, KD), "lnf_b": pp(ln_f_b, KD),
        "bv": np.ascontiguousarray(np.asarray(bv, f32)),
        "bo": np.ascontiguousarray(np.asarray(bo, f32)),
        "b2": np.ascontiguousarray(np.asarray(b2, f32)),
    }

    xr = np.asarray(x_refined, f32)
    xmp = np.zeros((B, T + 2 * HALO, D), f32)
    xmp[:, HALO:HALO + T] = np.asarray(x_mem, f32)

    in_maps = []
    for c in range(NCORES):
        b, ci = c // CPB, c % CPB
        r0 = ci * R
        # mask[kp, qt, kt, qi]: additive mask in S^T orientation
        qi = np.arange(P)
        kp = np.arange(P)
        t_g = r0 + np.arange(NR)[:, None] * P + qi[None, :]        # [qt, qi]
        # s_global = r0 - HALO + (qt+kt)*128 + kp
        s_g = (r0 - HALO + (np.arange(NR)[:, None, None] + np.arange(2)[None, :, None]) * P
               + kp[None, None, :])                                 # [qt, kt, kp]
        allowed = (np.abs(t_g[:, None, None, :] - s_g[:, :, :, None]) <= band) \
            & (s_g[:, :, :, None] >= 0) & (s_g[:, :, :, None] < T)  # [qt,kt,kp,qi]
        maskT = np.where(allowed, 0.0, NEG).astype(f32)
        maskT = np.ascontiguousarray(maskT.transpose(2, 0, 1, 3))   # [kp,qt,kt,qi]
        m = dict(common)
        m["xr"] = np.ascontiguousarray(xr[b, r0:r0 + R])
        m["xm"] = np.ascontiguousarray(xmp[b, r0:r0 + RH])
        m["maskT"] = maskT
        in_maps.append(m)
    return in_maps


def kernel(**inputs) -> np.ndarray:
    from concourse import bass_utils

    if "nc" not in _CACHE:
        _CACHE["nc"] = _build()
    nc = _CACHE["nc"]

    in_maps = _host_inputs(**inputs)
    res = bass_utils.run_bass_kernel_spmd(nc, in_maps,
                                          core_ids=list(range(NCORES)))
    out = np.empty((B, T, D), np.float32)
    for c in range(NCORES):
        b, ci = c // CPB, c % CPB
        out[b, ci * R:(ci + 1) * R] = res.results[c]["out"]
    return out
